# revision 72
# baseline (speedup 1.0000x reference)
"""Bass/Trainium2 kernel for nn_BasicQuantumAttention (B=4, L=2048, d=512, 8 cores).

Sharding: core (b, s) = batch b, stream s (real/imag); one program per
stream (each stream's own block-sparse keep-set: ~52/60 kept 128x128
tiles vs 79 for the mask union). Each core:

  - projects x[b] -> qT, kT (layout [d, L]) and v (layout [L, d]), all
    SBUF-resident. All heavy matmuls run as fp8 DoubleRow with hi/lo
    error compensation: every operand A is split into A_hi = e4m3(A)
    and A_lo = e5m2(A - A_hi) (e5m2 for the lo part because e4m3's
    subnormal floor flushes the residuals), and A@B is computed as
    three pure DoubleRow pass chains (Ah@Bh, Ah@Bl, Al@Bh). DoubleRow
    contracts two 128-K slabs per instruction at out_free x 0.5 cycles,
    so the three passes cost 0.75x the bf16 cycles while matching bf16
    accuracy (the dropped Al@Bl term is ~2^-8 relative). q/k are
    evicted from PSUM as (hi, lo) fp8 pairs so the score matmuls use
    the same scheme; v is evicted bf16 for the bf16 attnV matmuls.
    qs=0 runs as ct-pair-major waves (q, k, then v) so each wave's
    consumption rate stays under the serialized-DMA delivery rate; the
    sync stream is emitted in exact consumption order.
  - block-sparse masked attention with compile-time tile skipping at
    128x128 granularity on this stream's mask. Scores are fp8 tri-term
    DoubleRow; exp on ACT; all mask tiles are preloaded once into SBUF
    as fp8 (0/1 exact) to avoid per-job DMA latency. Per query block:
    the dps row-sum chain, then a DVE recip whose rank-1 PE broadcast
    (bps4, [128,4x128]) is deferred until after the attnV chains; the
    attnV PSUM result is normalized during its eviction by a single
    DVE mul with rb4, then split to (e4m3, e5m2) for the fp8 tri-term
    out-projection. Each block's PE-dense out-projection is deferred
    one job so its cross-engine ladder hides under the next job's
    attnV; the whole thing is software-pipelined one job ahead.
  - partial out-projection y^T_part = W_out^T[stream rows].T @ O_norm^T
    (fp8 tri-term DoubleRow when biases are zero, bf16 otherwise).
Host sums the two per-stream partial y^T per batch and untransposes.
"""
import sys

sys.path.insert(0, "/opt/trn_rl_repo")

import numpy as np
import ml_dtypes

import concourse.bass as bass
import concourse.tile as tile
from concourse import bacc, mybir
from concourse.bass_utils import run_bass_kernel_spmd

B, L, D = 4, 2048, 512
C6 = 6 * D            # 3072 input features
CT = C6 // 128        # 24 contraction tiles
QS = L // 512         # 4 query slices of 512 (normalization/out-proj grain)
QB = L // 128         # 16 query blocks of 128 (attention grain)
KT = L // 128         # 16 key tiles of 128
F32 = mybir.dt.float32
BF16 = mybir.dt.bfloat16
F8H = mybir.dt.float8e4
F8L = mybir.dt.float8e5
F16 = mybir.dt.float16
DR = mybir.MatmulPerfMode.DoubleRow
SCALE = float(D) ** -0.5
BF = ml_dtypes.bfloat16
H8 = ml_dtypes.float8_e4m3
E5 = ml_dtypes.float8_e5m2

# feature offsets inside qkv = [q_r q_i k_r k_i v_r v_i] (each D wide)
_Q_OFF = {0: 0 * D, 1: 1 * D}
_K_OFF = {0: 2 * D, 1: 3 * D}
_V_OFF = {0: 4 * D, 1: 5 * D}

LAST_RESULTS = None   # list of per-stream BassKernelResults
LAST_PROGRAMS = None  # list of per-stream compiled Bacc programs


def build_program(kept, needs_mask, slot_of, n_slots, zero_bias=False):
    """kept: {qb_global: [kt,...]} keep lists at 128x128 granularity for
    THIS stream; needs_mask: set[(qb,kt)]; slot_of: {(qb,kt): slot}."""
    nc = bacc.Bacc(None, target_bir_lowering=False, debug=False)

    x8_t = nc.dram_tensor("x8_t", [CT, 128, L], F8H, kind="ExternalInput")
    xl_t = nc.dram_tensor("xl_t", [CT, 128, L], F8L, kind="ExternalInput")
    wqkh_t = nc.dram_tensor("wqkh_t", [CT, 128, 2, 512], F8H, kind="ExternalInput")
    wqkl_t = nc.dram_tensor("wqkl_t", [CT, 128, 2, 512], F8L, kind="ExternalInput")
    wvh_t = nc.dram_tensor("wvh_t", [CT, 128, 512], F8H, kind="ExternalInput")
    wvl_t = nc.dram_tensor("wvl_t", [CT, 128, 512], F8L, kind="ExternalInput")
    w_o = nc.dram_tensor("w_o", [4, 128, 2 * D], BF16, kind="ExternalInput")
    woh_t = nc.dram_tensor("woh_t", [4, 128, 2 * D], F8H, kind="ExternalInput")
    wol_t = nc.dram_tensor("wol_t", [4, 128, 2 * D], F8L, kind="ExternalInput")
    b_qk = nc.dram_tensor("b_qk", [128, 8], F32, kind="ExternalInput")
    b_y = nc.dram_tensor("b_y", [128, 8], F32, kind="ExternalInput")
    mask_t = nc.dram_tensor("mask_t", [n_slots, 128, 128], F8H, kind="ExternalInput")
    ones_a = nc.dram_tensor("ones_a", [128, 1], BF16, kind="ExternalInput")
    ones_b = nc.dram_tensor("ones_b", [1, 128], BF16, kind="ExternalInput")
    b_yr = nc.dram_tensor("b_yr", [1, 2 * D], BF16, kind="ExternalInput")
    y = nc.dram_tensor("y", [2 * D, L], F32, kind="ExternalOutput")

    with tile.TileContext(nc) as tc, \
         nc.allow_low_precision(reason="fp8 hi/lo compensated matmuls"):
        with tc.tile_pool(name="consts", bufs=1) as consts, \
             tc.tile_pool(name="kqv", bufs=1) as kqv:
            ones_k = consts.tile([128, 1], BF16)
            ones_1 = consts.tile([1, 128], BF16)
            bqk_s = consts.tile([128, 8], F32)
            by_s = consts.tile([128, 8], F32)
            byr_s = consts.tile([1, 2 * D], BF16)
            if zero_bias:
                woh_sb = consts.tile([128, 4, 2 * D], F8H)
                wol_sb = consts.tile([128, 4, 2 * D], F8L)
            else:
                wo_sb = consts.tile([128, 4, 2 * D], BF16)
            # all mask tiles live in SBUF for the whole program (fp8: 0/1 is
            # exact), loaded with ONE transfer during the projection phase —
            # per-job mask DMAs would serialize on HWDGE (625ns each) and
            # add ~1.5us of latency in front of every masked attnV
            mask_sb = consts.tile([128, n_slots, 128], F8H)

            qh_sb = kqv.tile([128, 4, L], F8H)
            ql_sb = kqv.tile([128, 4, L], F8L)
            kh_sb = kqv.tile([128, 4, L], F8H)
            kl_sb = kqv.tile([128, 4, L], F8L)
            v_sb = kqv.tile([128, KT, 512], BF16)

            # ---------------- projection phase ----------------
            with tc.tile_pool(name="wc", bufs=1) as wcp, \
                 tc.tile_pool(name="xin", bufs=2) as xp, \
                 tc.tile_pool(name="ev", bufs=4) as evp, \
                 tc.tile_pool(name="pp", bufs=8, space="PSUM") as pp:
                wqkh_sb = wcp.tile([128, CT, 2, 512], F8H)
                wqkl_sb = wcp.tile([128, CT, 2, 512], F8L)
                wvh_sb = wcp.tile([128, CT, 512], F8H)
                wvl_sb = wcp.tile([128, CT, 512], F8L)

                x_tiles = {}

                def load_x(qs_, pieces=1):
                    xh = xp.tile([128, CT, 512], F8H, name=f"x8{qs_}", tag="x8")
                    xl = xp.tile([128, CT, 512], F8L, name=f"xl{qs_}", tag="xl")
                    sl = slice(qs_ * 512, (qs_ + 1) * 512)
                    step = CT // pieces
                    for c0 in range(0, CT, step):
                        cs = slice(c0, c0 + step)
                        nc.sync.dma_start(
                            out=xh[:, cs, :],
                            in_=x8_t[cs, :, sl].rearrange("ct p n -> p ct n"))
                        nc.sync.dma_start(
                            out=xl[:, cs, :],
                            in_=xl_t[cs, :, sl].rearrange("ct p n -> p ct n"))
                    x_tiles[qs_] = (xh, xl)

                # PE p-state warm-up: burn the 0.65->2.4GHz ramp on dummy
                # matmuls while the first input DMAs are in flight
                warm = consts.tile([128, 128], BF16)
                nc.gpsimd.memset(warm, 0.0)
                wps = pp.tile([128, 512], F32, name="wps", tag="ps")
                for _ in range(12):
                    nc.tensor.matmul(wps[:, 0:128], warm[:, :], warm[:, :],
                                     start=True, stop=True)

                x0h = xp.tile([128, CT, 512], F8H, name="x80", tag="x8")
                x0l = xp.tile([128, CT, 512], F8L, name="xl0", tag="xl")
                # qs=0 runs wave1 = q+k (8 chains, consuming wqk hi/lo + x
                # hi/lo at ~1.16us/ct delivered vs 1.28us/ct consumed) then
                # wave2 = v (wv hi/lo, delivered during wave1's tail). The
                # DMA engines are one serialized resource, so the sync
                # stream is ordered exactly in consumption order; only the
                # first x8 piece + consts ride gpsimd for issue parallelism.
                nc.gpsimd.dma_start(out=x0h[:, 0:2, :],
                                    in_=x8_t[0:2, :, 0:512].rearrange(
                                        "ct p n -> p ct n"))
                nc.sync.dma_start(
                    out=wqkh_sb[:, 0:2, :, :],
                    in_=wqkh_t[0:2].rearrange("ct p f d -> p ct f d"))
                nc.sync.dma_start(
                    out=wqkl_sb[:, 0:2, :, :],
                    in_=wqkl_t[0:2].rearrange("ct p f d -> p ct f d"))
                nc.sync.dma_start(out=x0l[:, 0:2, :],
                                  in_=xl_t[0:2, :, 0:512].rearrange(
                                      "ct p n -> p ct n"))
                # consts trail on gpsimd (nothing needs them until the first
                # evictions ~15us in)
                nc.gpsimd.dma_start(out=ones_k, in_=ones_a[:, :])
                nc.gpsimd.dma_start(out=ones_1, in_=ones_b[:, :])
                nc.gpsimd.dma_start(out=bqk_s, in_=b_qk[:, :])
                nc.gpsimd.dma_start(out=by_s, in_=b_y[:, :])
                nc.gpsimd.dma_start(out=byr_s, in_=b_yr[:, :])
                # pre-warm the exp activation table while PE projects
                scrap = consts.tile([128, 8], BF16)
                nc.scalar.activation(out=scrap, in_=bqk_s,
                                     func=mybir.ActivationFunctionType.Exp)
                ct_groups = [[2, 3], [4, 5], [6, 7]] + \
                    [list(range(c, min(c + 4, CT))) for c in range(8, CT, 4)]
                for grp_ in ct_groups:
                    c0, cn = grp_[0], len(grp_)
                    nc.sync.dma_start(
                        out=wqkh_sb[:, c0:c0 + cn, :, :],
                        in_=wqkh_t[c0:c0 + cn].rearrange("ct p f d -> p ct f d"))
                    nc.sync.dma_start(
                        out=wqkl_sb[:, c0:c0 + cn, :, :],
                        in_=wqkl_t[c0:c0 + cn].rearrange("ct p f d -> p ct f d"))
                    nc.sync.dma_start(
                        out=x0h[:, c0:c0 + cn, :],
                        in_=x8_t[c0:c0 + cn, :, 0:512].rearrange(
                            "ct p n -> p ct n"))
                    nc.sync.dma_start(
                        out=x0l[:, c0:c0 + cn, :],
                        in_=xl_t[c0:c0 + cn, :, 0:512].rearrange(
                            "ct p n -> p ct n"))
                # wv hi/lo stream for wave2, in consumption order
                for c0 in range(0, CT, 6):
                    cn = min(6, CT - c0)
                    nc.sync.dma_start(
                        out=wvh_sb[:, c0:c0 + cn, :],
                        in_=wvh_t[c0:c0 + cn].rearrange("ct p d -> p ct d"))
                    nc.sync.dma_start(
                        out=wvl_sb[:, c0:c0 + cn, :],
                        in_=wvl_t[c0:c0 + cn].rearrange("ct p d -> p ct d"))
                # one-shot mask preload (needed from the first attention job)
                nc.sync.dma_start(out=mask_sb,
                                  in_=mask_t.rearrange("s p n -> p s n"))
                x_tiles[0] = (x0h, x0l)
                # x[1] is consumed right as its delivery completes: split it
                # into pieces so the sem fires progressively
                load_x(1, pieces=3)

                def evict(kind, ft, ps, qs_):
                    if kind == "v":
                        # v bias is folded into b_y on the host (as in the
                        # bf16 kernel), so v eviction is always a plain copy
                        nc.scalar.copy(out=v_sb[:, qs_ * 4 + ft, :], in_=ps)
                        return
                    hi, lo = (qh_sb, ql_sb) if kind == "q" else (kh_sb, kl_sb)
                    bi = ft if kind == "q" else 4 + ft
                    sl = slice(qs_ * 512, (qs_ + 1) * 512)
                    if zero_bias:
                        nc.scalar.copy(out=hi[:, ft, sl], in_=ps)
                        nc.vector.tensor_sub(lo[:, ft, sl], ps, hi[:, ft, sl])
                    else:
                        tmp = evp.tile([128, 512], BF16, name="evt", tag="evt")
                        nc.scalar.activation(
                            out=tmp, in_=ps,
                            func=mybir.ActivationFunctionType.Identity,
                            bias=bqk_s[:, bi:bi + 1])
                        nc.scalar.copy(out=hi[:, ft, sl], in_=tmp)
                        nc.vector.tensor_sub(lo[:, ft, sl], tmp, hi[:, ft, sl])

                def mm_steps(kind, ft, c, xt):
                    """The 3 DoubleRow (lhsT, rhs) pairs for ct-pair c."""
                    xh, xl = xt
                    cp = slice(c, c + 2)
                    fsl = slice(ft * 128, (ft + 1) * 128)
                    if kind == "v":
                        return [(xh[:, cp, fsl], wvh_sb[:, cp, :]),
                                (xh[:, cp, fsl], wvl_sb[:, cp, :]),
                                (xl[:, cp, fsl], wvh_sb[:, cp, :])]
                    fc = 0 if kind == "q" else 1
                    return [(wqkh_sb[:, cp, fc, fsl], xh[:, cp, :]),
                            (wqkl_sb[:, cp, fc, fsl], xh[:, cp, :]),
                            (wqkh_sb[:, cp, fc, fsl], xl[:, cp, :])]

                NP = 3 * (CT // 2)  # matmuls per chain

                # qs=0: ct-pair-major waves so PE consumption tracks DMA
                # delivery; wave1 = q+k (needs wqk+x, 8 PSUM banks),
                # wave2 = v (wv lands during wave1)
                x0 = x_tiles.pop(0)
                waves = [[("q", ft) for ft in range(4)],
                         [("k", ft) for ft in range(4)],
                         [("v", nt) for nt in range(4)]]
                for wave in waves:
                    pss = {u: pp.tile([128, 512], F32, name=f"ps{u[0]}{u[1]}",
                                      tag="ps") for u in wave}
                    cnt = {u: 0 for u in wave}
                    for c in range(0, CT, 2):
                        for u in wave:
                            for lhsT, rhs in mm_steps(u[0], u[1], c, x0):
                                nc.tensor.matmul(
                                    pss[u][:, :], lhsT, rhs,
                                    start=(cnt[u] == 0),
                                    stop=(cnt[u] == NP - 1), perf_mode=DR)
                                cnt[u] += 1
                    for u in wave:
                        evict(u[0], u[1], pss[u], 0)

                for qs in range(1, QS):
                    if qs + 1 < QS:
                        load_x(qs + 1)
                    x_qs = x_tiles.pop(qs)
                    for kind in ("q", "k", "v"):
                        for ft in range(4):
                            ps = pp.tile([128, 512], F32, name="ps", tag="ps")
                            i = 0
                            for c in range(0, CT, 2):
                                for lhsT, rhs in mm_steps(kind, ft, c, x_qs):
                                    nc.tensor.matmul(
                                        ps[:, :], lhsT, rhs,
                                        start=(i == 0), stop=(i == NP - 1),
                                        perf_mode=DR)
                                    i += 1
                            evict(kind, ft, ps, qs)

            # ---------------- attention + out-projection ----------------
            jobs = []   # (qs, qb, [groups of up to 4 kt])
            for qs in range(QS):
                order = list(range(4))
                for qb in order:
                    klist = kept[qs * 4 + qb]
                    jobs.append((qs, qb,
                                 [klist[i:i + 4]
                                  for i in range(0, len(klist), 4)]))

            with tc.tile_pool(name="sy", bufs=2, space="PSUM") as syp, \
                 tc.tile_pool(name="op", bufs=2, space="PSUM") as opp, \
                 tc.tile_pool(name="dn", bufs=1, space="PSUM") as dnp, \
                 tc.tile_pool(name="yp", bufs=3, space="PSUM") as ypp, \
                 tc.tile_pool(name="pt", bufs=8) as ptp, \
                 tc.tile_pool(name="ot", bufs=3) as otp, \
                 tc.tile_pool(name="ot8", bufs=4) as ot8p, \
                 tc.tile_pool(name="sm", bufs=3) as smp, \
                 tc.tile_pool(name="yo", bufs=4) as yop:
                if zero_bias:
                    nc.scalar.dma_start(
                        out=woh_sb, in_=woh_t.rearrange("ft p g -> p ft g"))
                    nc.scalar.dma_start(
                        out=wol_sb, in_=wol_t.rearrange("ft p g -> p ft g"))
                else:
                    nc.scalar.dma_start(
                        out=wo_sb, in_=w_o.rearrange("ft p g -> p ft g"))

                state = {}   # per-qs tiles: ot, dps

                def emit_scores_grp(qs, qb, grp):
                    g = qs * 4 + qb
                    w = len(grp)
                    gsl = slice(g * 128, (g + 1) * 128)
                    sps = syp.tile([128, 4, 128], F32, name="sps", tag="sps")
                    for j, kt in enumerate(grp):
                        ksl = slice(kt * 128, (kt + 1) * 128)
                        steps = []
                        for dt in (0, 2):
                            steps.append((kh_sb[:, dt:dt + 2, ksl],
                                          qh_sb[:, dt:dt + 2, gsl]))
                        for dt in (0, 2):
                            steps.append((kh_sb[:, dt:dt + 2, ksl],
                                          ql_sb[:, dt:dt + 2, gsl]))
                        for dt in (0, 2):
                            steps.append((kl_sb[:, dt:dt + 2, ksl],
                                          qh_sb[:, dt:dt + 2, gsl]))
                        for i, (lhsT, rhs) in enumerate(steps):
                            nc.tensor.matmul(
                                sps[:, j, :], lhsT, rhs,
                                start=(i == 0), stop=(i == len(steps) - 1),
                                perf_mode=DR)
                    pT = ptp.tile([128, 4, 128], BF16, name="pT", tag="pT")
                    nc.scalar.activation(
                        out=pT[:, :w, :], in_=sps[:, :w, :],
                        func=mybir.ActivationFunctionType.Exp, scale=SCALE)
                    masked = [j for j, kt in enumerate(grp)
                              if (g, kt) in needs_mask]
                    if masked:
                        slots = [slot_of[(g, grp[j])] for j in masked]
                        contig = (len(masked) == masked[-1] - masked[0] + 1
                                  and slots == list(range(slots[0],
                                                          slots[0] + len(slots))))
                        if contig:
                            j0, sw = masked[0], len(masked)
                            nc.vector.tensor_mul(
                                pT[:, j0:j0 + sw, :], pT[:, j0:j0 + sw, :],
                                mask_sb[:, slots[0]:slots[0] + sw, :])
                        else:
                            for i, j in enumerate(masked):
                                nc.vector.tensor_mul(
                                    pT[:, j, :], pT[:, j, :],
                                    mask_sb[:, slots[i], :])
                    return pT

                def emit_qb_recip(qs, qb):
                    """recip + widened recs (DVE-only), emitted right
                    after the dps chain so it lands while PE runs the attnV
                    chains; the PE-side broadcast happens in emit_qb_bcast
                    AFTER the chains so PE never head-of-line-waits on
                    DVE."""
                    dps = state[("dps", qs)]
                    c0, c1 = qb * 128, (qb + 1) * 128
                    recf = smp.tile([1, 128], F32, tag="recf", name="recf")
                    nc.vector.reciprocal(recf, dps[:, c0:c1])
                    recs4 = smp.tile([1, 512], BF16, tag="recs", name="recs4")
                    for r in range(4):
                        nc.vector.tensor_copy(
                            out=recs4[:, r * 128:(r + 1) * 128], in_=recf)
                    return recs4

                def emit_qb_bcast(recs4):
                    bps4 = ypp.tile([128, 4, 128], F32, tag="yp", name="bps4")
                    nc.tensor.matmul(bps4[:, :, :], ones_1[:, :], recs4[:, :],
                                     start=True, stop=True)
                    rb4 = smp.tile([128, 4, 128], BF16, tag="rb4", name="rb4")
                    nc.scalar.copy(out=rb4, in_=bps4)
                    return rb4

                def emit_qb_split(ot, qb):
                    """hi/lo split of the normalized O block (fp8 out-proj
                    operands)."""
                    if not zero_bias:
                        return None, None
                    c0, c1 = qb * 128, (qb + 1) * 128
                    oth = ot8p.tile([128, 4, 128], F8H, name="othq",
                                    tag="othq")
                    otl = ot8p.tile([128, 4, 128], F8L, name="otlq",
                                    tag="otlq")
                    nc.scalar.copy(out=oth, in_=ot[:, :, c0:c1])
                    nc.vector.tensor_sub(otl, ot[:, :, c0:c1], oth)
                    return oth, otl

                def emit_qb_head(ot, qs, qb, oth, otl, last):
                    """out-project ONE query block (PE-dense; deferred one
                    job so its ladder inputs are long ready)."""
                    c0, c1 = qb * 128, (qb + 1) * 128
                    for gh in range(2):
                        ypsg = ypp.tile([128, 4, 128], F32, tag="yp", name="ypsg")
                        for gi in range(4):
                            gt = gh * 4 + gi
                            gsl = slice(gt * 128, (gt + 1) * 128)
                            if zero_bias:
                                steps = []
                                for ft in (0, 2):
                                    steps.append((woh_sb[:, ft:ft + 2, gsl],
                                                  oth[:, ft:ft + 2, :]))
                                for ft in (0, 2):
                                    steps.append((woh_sb[:, ft:ft + 2, gsl],
                                                  otl[:, ft:ft + 2, :]))
                                for ft in (0, 2):
                                    steps.append((wol_sb[:, ft:ft + 2, gsl],
                                                  oth[:, ft:ft + 2, :]))
                                for i, (lh, rh) in enumerate(steps):
                                    nc.tensor.matmul(
                                        ypsg[:, gi, :], lh, rh,
                                        start=(i == 0), stop=(i == 5),
                                        perf_mode=DR)
                            else:
                                for ft in range(4):
                                    nc.tensor.matmul(
                                        ypsg[:, gi, :],
                                        wo_sb[:, ft, gsl],
                                        ot[:, ft, c0:c1],
                                        start=(ft == 0), stop=False)
                                nc.tensor.matmul(
                                    ypsg[:, gi, :],
                                    byr_s[:, gsl],
                                    ones_1[:, :], start=False, stop=True)
                        y_sbh = yop.tile([128, 4, 128], F32,
                                         name="ysq", tag="ysq")
                        if gh == 1:
                            nc.vector.tensor_copy(out=y_sbh, in_=ypsg)
                        else:
                            nc.scalar.copy(out=y_sbh, in_=ypsg)
                        q_ = nc.sync if gh == 0 else nc.scalar
                        q_.dma_start(
                            out=y[gh * 512:(gh + 1) * 512,
                                  qs * 512 + c0:qs * 512 + c1].rearrange(
                                      "(a p) n -> p a n", p=128),
                            in_=y_sbh)

                def emit_attnv_job(job, pts):
                    qs, qb, groups = job
                    dps = state[("dps", qs)]
                    n = sum(len(g) for g in groups)
                    i = 0
                    for gi, grp in enumerate(groups):
                        for j, kt in enumerate(grp):
                            nc.tensor.matmul(
                                dps[:, qb * 128:(qb + 1) * 128],
                                ones_k[:, :], pts[gi][:, j, :],
                                start=(i == 0), stop=(i == n - 1))
                            i += 1
                    recs4 = emit_qb_recip(qs, qb)
                    ops = opp.tile([128, 4, 128], F32, name="ops", tag="ops")
                    for dvt in range(4):
                        i = 0
                        for gi, grp in enumerate(groups):
                            for j, kt in enumerate(grp):
                                nc.tensor.matmul(
                                    ops[:, dvt, :],
                                    v_sb[:, kt, dvt * 128:(dvt + 1) * 128],
                                    pts[gi][:, j, :],
                                    start=(i == 0), stop=(i == n - 1))
                                i += 1
                    rb4 = emit_qb_bcast(recs4)
                    ot = state[("ot", qs)]
                    # normalize during the PSUM eviction: one DVE mul
                    # replaces the eviction copy + per-ft rb muls
                    nc.vector.tensor_mul(
                        ot[:, :, qb * 128:(qb + 1) * 128], ops, rb4)
                    oth, otl = emit_qb_split(ot, qb)
                    return lambda last=False: emit_qb_head(
                        ot, qs, qb, oth, otl, last)

                # pipeline: scores(i+1) ahead of attnV(i); each qb head
                # (normalize/split/out-proj) deferred ONE more job so its
                # ladder hides under the next job's attnV PE work
                prev = None
                pend_head = None
                for ji, job in enumerate(jobs):
                    qs, qb, groups = job
                    if ("ot", qs) not in state:
                        state[("ot", qs)] = otp.tile(
                            [128, 4, 512], BF16, name="ot", tag="ot")
                        state[("dps", qs)] = dnp.tile(
                            [1, 512], F32, name="dps", tag="dps")
                    pts = [emit_scores_grp(qs, qb, grp) for grp in groups]
                    if prev is not None:
                        h = emit_attnv_job(*prev)
                        if pend_head is not None:
                            pend_head()
                        pend_head = h
                    prev = (job, pts)
                h_last = emit_attnv_job(*prev)
                if pend_head is not None:
                    pend_head()
                h_last(True)

    nc.compile()
    return nc


def _prep_mask(mask):
    """Compile-time 128x128 tile analysis for ONE stream's mask."""
    mt = np.ascontiguousarray(np.asarray(mask).T)
    kept = {}
    needs_mask = set()
    slot_of = {}
    slots = []  # (qb, kt)
    for g in range(QB):
        klist = []
        for kt in range(KT):
            sub = mt[kt * 128:(kt + 1) * 128, g * 128:(g + 1) * 128]
            if not sub.any():
                continue
            klist.append(kt)
            if not sub.all():
                needs_mask.add((g, kt))
                slot_of[(g, kt)] = len(slots)
                slots.append((g, kt))
        kept[g] = klist
    n_slots = max(1, len(slots))
    md = np.zeros((n_slots, 128, 128), H8)
    for i, (g, kt) in enumerate(slots):
        md[i] = mt[kt * 128:(kt + 1) * 128,
                   g * 128:(g + 1) * 128].astype(H8)
    return kept, needs_mask, slot_of, n_slots, md


def _split8(a):
    h = a.astype(H8)
    l = (a - h.astype(np.float32)).astype(E5)
    return h, l


def kernel(q_real, q_imag, k_real, k_imag, v_real, v_imag,
           W_qkv, b_qkv, W_out, b_out, mask_real, mask_imag, _trace=False):
    global LAST_RESULTS, LAST_PROGRAMS
    args = [np.asarray(a) for a in (q_real, q_imag, k_real, k_imag, v_real, v_imag)]
    W_qkv = np.asarray(W_qkv, np.float32)
    b_qkv = np.asarray(b_qkv, np.float32)
    W_out = np.asarray(W_out, np.float32)
    b_out = np.asarray(b_out, np.float32)

    zb = bool(not b_qkv.any() and not b_out.any())
    preps = [_prep_mask(mask_real), _prep_mask(mask_imag)]
    programs = [build_program(*p[:4], zero_bias=zb) for p in preps]

    # x^T per batch, c-tiled hi/lo: [CT, 128, L] e4m3 + e5m2
    x8_ts, xl_ts = [], []
    for b in range(B):
        xb = np.concatenate([a[b] for a in args], axis=1)          # [L, 6D]
        xt = np.ascontiguousarray(xb.T.astype(np.float32))          # [6D, L]
        xh, xl = _split8(xt)
        x8_ts.append(np.ascontiguousarray(xh.reshape(CT, 128, L)))
        xl_ts.append(np.ascontiguousarray(xl.reshape(CT, 128, L)))

    W6T = W_qkv.T  # [c, f]
    W2T = W_out.T  # [f=2D, g=2D]
    stream_inputs = []
    for s in range(2):
        wq = W6T[:, _Q_OFF[s]:_Q_OFF[s] + D].reshape(CT, 128, 512)
        wk = W6T[:, _K_OFF[s]:_K_OFF[s] + D].reshape(CT, 128, 512)
        wv = W6T[:, _V_OFF[s]:_V_OFF[s] + D].reshape(CT, 128, 512)
        wqk = np.ascontiguousarray(np.stack([wq, wk], axis=2))     # [CT,128,2,512]
        wqkh, wqkl = _split8(wqk)
        wvh, wvl = _split8(np.ascontiguousarray(wv))
        bq = b_qkv[_Q_OFF[s]:_Q_OFF[s] + D].reshape(4, 128).T
        bk = b_qkv[_K_OFF[s]:_K_OFF[s] + D].reshape(4, 128).T
        b_qks = np.ascontiguousarray(
            np.concatenate([bq, bk], axis=1), dtype=np.float32)    # [128, 8]
        wo_r = W2T[s * D:(s + 1) * D, :].reshape(4, 128, 2 * D)
        w_os = np.ascontiguousarray(wo_r.astype(BF))
        woh, wol = _split8(np.ascontiguousarray(wo_r))
        if s == 0:
            b_v_cat = np.concatenate([b_qkv[_V_OFF[0]:_V_OFF[0] + D],
                                      b_qkv[_V_OFF[1]:_V_OFF[1] + D]])
            b_eff = (W_out @ b_v_cat + b_out).astype(np.float32)
            b_ys = np.ascontiguousarray(b_eff.reshape(8, 128).T)
        else:
            b_ys = np.zeros((128, 8), np.float32)
        stream_inputs.append(dict(
            wqkh=np.ascontiguousarray(wqkh), wqkl=np.ascontiguousarray(wqkl),
            wvh=np.ascontiguousarray(wvh), wvl=np.ascontiguousarray(wvl),
            b_qks=b_qks, w_os=w_os, b_ys=b_ys,
            woh=np.ascontiguousarray(woh), wol=np.ascontiguousarray(wol)))

    LAST_RESULTS = []
    LAST_PROGRAMS = programs
    stream_res = []
    for s in range(2):
        si = stream_inputs[s]
        in_maps = []
        for b in range(B):
            in_maps.append({
                "x8_t": x8_ts[b], "xl_t": xl_ts[b],
                "wqkh_t": si["wqkh"], "wqkl_t": si["wqkl"],
                "wvh_t": si["wvh"], "wvl_t": si["wvl"],
                "w_o": si["w_os"], "woh_t": si["woh"], "wol_t": si["wol"],
                "b_qk": si["b_qks"], "b_y": si["b_ys"],
                "b_yr": np.ascontiguousarray(
                    si["b_ys"].T.reshape(1, 2 * D).astype(BF)),
                "mask_t": preps[s][4],
                "ones_a": np.ones((128, 1), BF),
                "ones_b": np.ones((1, 128), BF),
            })
        res = run_bass_kernel_spmd(programs[s], in_maps,
                                   core_ids=[4 * s + b for b in range(B)],
                                   trace=_trace)
        LAST_RESULTS.append(res)
        stream_res.append(res)

    out_real = np.empty((B, L, D), np.float32)
    out_imag = np.empty((B, L, D), np.float32)
    for b in range(B):
        yt = stream_res[0].results[b]["y"] + stream_res[1].results[b]["y"]
        yb = yt.T                                                   # [L, 2D]
        out_real[b] = yb[:, :D]
        out_imag[b] = yb[:, D:]
    return out_real, out_imag
